# revision 6
# baseline (speedup 1.0000x reference)
"""Trainium2 Bass kernel for the 3-layer LIF spiking net (nn_Net_70927089926628).

Reference semantics per timestep t:
    cur1 = x_t * W_in.T + b_in            [B,H]
    m1   = b1*m1 + cur1 - thr1*s1_prev    (reset mask == previous spike)
    s1   = (m1 > thr1)
    cur2 = s1 @ W_h.T + b_h
    m2   = b2*m2 + cur2 - thr2*s2_prev
    s2   = (m2 > thr2)
    m3   = b3*m3 + s2 @ W_out.T + b_out   -> recorded every step (the output)

Mapping:
  - batch 2048 sharded 8 ways -> B=256 per core; params replicated.
  - state feature-major [H=128 partitions, B=256 free].
  - membranes live in PSUM in b^-j scaled form: P_j = b^-j * m_(t0+j) within a
    block of K_BLK steps; every matmul contribution at local step j is
    prescaled by b^-j (folded into host-precomputed stationary operands), so
    the per-step decay multiply disappears; one per-partition rescale op per
    block renormalizes (P *= b^K_BLK).
  - all stationary operands are split into bf16 terms (3 terms ~ exact fp32);
    moving operands are spikes {0,1} (exact in bf16) or split x rows.
  - spikes via DVE tensor_scalar is_gt against c_j = thr * b^-j.
  - layer-3: sliding-window matmul collects cur3 rows into a PSUM tile
    (partition = timestep mod 128); final scan m3 = L @ C as blocked
    lower-triangular fp32 matmuls, + closed-form b_out bias.
"""
import sys
import numpy as np
from contextlib import ExitStack

sys.path.insert(0, '/opt/trn_rl_repo')
if '/root/problem' not in sys.path:
    sys.path.insert(0, '/root/problem')

import concourse.bass as bass
import concourse.tile as tile
import concourse.mybir as mybir
from concourse import bass_utils

F32 = mybir.dt.float32
BF16 = mybir.dt.bfloat16
AOP = mybir.AluOpType
AFT = mybir.ActivationFunctionType

H = 128
NCORES = 8
K_BLK = 8                          # rescale block
B_CLAMP = 1e-5                     # lower clamp on decay factors

# number of bf16 split terms per path (3 ~= exact fp32)
NT_W2 = 3
NT_R1 = 3
NT_R2 = 3
NT_ZW = 2

# cur1 combo: (A-term, x-row) pairs; x rows 0=xhi 1=xmid 2=xlo; 'b' rows pair
# with ones. Order defines both lhsT rows and x staging rows.
CUR1_PAIRS = [(0, 0), (0, 1), (1, 0), (0, 2), (1, 1), (2, 0),
              ('b', 0), ('b', 1), ('b', 2)]
XSTAGE_ROWS = [0, 1, 0, 2, 1, 0, 3, 3, 3]   # source row per staged row (3=ones)
NXROW = len(XSTAGE_ROWS)                    # 9


def _split_bf16(a, nterms):
    import ml_dtypes
    out = []
    r = np.asarray(a, np.float32)
    for _ in range(nterms):
        hi = r.astype(ml_dtypes.bfloat16)
        out.append(hi.astype(np.float32))
        r = (r - out[-1]).astype(np.float32)
    return out


def _geom_bias(b_out, b3, T):
    t = np.arange(1, T + 1, dtype=np.float64)
    if abs(1.0 - b3) < 1e-12:
        s = t.astype(np.float64)
    else:
        s = (1.0 - b3 ** t) / (1.0 - b3)
    return (b_out * s).astype(np.float32)


class Prep:
    """Host-side precomputation (shared by all cores)."""

    def __init__(self, W_in, b_in, beta_in, thr_in, W_h, b_h, beta_h, thr_h,
                 W_out, b_out, beta_out, T):
        f64 = np.float64
        self.T = T
        self.CB = min(128, T)              # layer-3 collection block
        self.nblk = (T + self.CB - 1) // self.CB
        k = K_BLK
        b1 = np.clip(beta_in.astype(f64), B_CLAMP, 1.0)
        b2 = np.clip(beta_h.astype(f64), B_CLAMP, 1.0)
        b3 = float(np.clip(beta_out.astype(f64), 0.0, 1.0)[0])
        thr1 = thr_in.astype(f64)
        thr2 = thr_h.astype(f64)
        win = W_in[:, 0].astype(f64)
        wout = W_out[0, :].astype(f64)

        s1 = np.stack([b1 ** (-j) for j in range(k)])      # [k,H]
        s2 = np.stack([b2 ** (-j) for j in range(k)])

        # cur1 combo lhsT_j [NXROW, H]
        self.cur1_lhsT = np.zeros((k, NXROW, H), np.float32)
        for j in range(k):
            A_t = _split_bf16((win * s1[j]).astype(np.float32), 3)
            B_t = _split_bf16((b_in.astype(f64) * s1[j]).astype(np.float32), 3)
            for r, (a, xi) in enumerate(CUR1_PAIRS):
                self.cur1_lhsT[j, r] = B_t[xi] if a == 'b' else A_t[a]

        # W2_j: lhsT[k=h1, m=h2] = W_h[h2,h1] * s2_j[h2]
        W2 = W_h.astype(f64).T[None, :, :] * s2[:, None, :]   # [k, h1, h2]
        self.W2_t = []
        for j in range(k):
            self.W2_t.append(_split_bf16(W2[j].astype(np.float32), NT_W2))
        # reset diags
        self.d1_t, self.d2_t = [], []
        for j in range(k):
            self.d1_t.append([np.diag(v) for v in _split_bf16(
                (-(thr1 * s1[j])).astype(np.float32), NT_R1)])
            self.d2_t.append([np.diag(v) for v in _split_bf16(
                (-(thr2 * s2[j])).astype(np.float32), NT_R2)])
        # bias2_j rows [3, H]: split of b_h * s2_j (matmul against 3 ones rows)
        self.bias2_lhsT = np.stack(
            [np.stack(_split_bf16((b_h.astype(f64) * s2[j]).astype(np.float32), 3))
             for j in range(k)])

        self.c1 = (thr1[None, :] * s1).astype(np.float32).T   # [H, k]
        self.c2 = (thr2[None, :] * s2).astype(np.float32).T
        self.r1 = (b1 ** k).astype(np.float32)[:, None]       # [H,1]
        self.r2 = (b2 ** k).astype(np.float32)[:, None]

        # layer-3 Z buffers [H, 2*CB-1] with W_out term at col CB-1
        CB = self.CB
        wout_t = _split_bf16(wout.astype(np.float32), NT_ZW)
        self.Z_t = []
        for i in range(NT_ZW):
            Z = np.zeros((H, 2 * CB - 1), np.float32)
            Z[:, CB - 1] = wout_t[i]
            self.Z_t.append(Z)

        # L-scan matrices (fp32) [CB, CB]
        idx = np.arange(CB)
        dt_ = idx[None, :] - idx[:, None]                     # t - tau
        with np.errstate(over='ignore', under='ignore'):
            LD = np.where(dt_ >= 0, b3 ** np.maximum(dt_, 0), 0.0)
        self.LD = LD.astype(np.float32)
        self.LF = []
        for d in range(1, self.nblk):
            with np.errstate(over='ignore', under='ignore'):
                M = b3 ** (dt_.astype(f64) + CB * d)
            M = np.where(np.isfinite(M), M, 0.0).astype(np.float32)
            self.LF.append(None if np.abs(M).max() < 1e-37 else M)
        self.l_bias = _geom_bias(float(np.asarray(b_out).ravel()[0]), b3,
                                 T).reshape(self.nblk, self.CB)

    def pack_params(self):
        import ml_dtypes
        k = K_BLK
        bf_cols, off_bf = [], {}

        def add_bf(name, arr2d):
            rows, C = arr2d.shape
            off_bf[name] = (sum(c.shape[1] for c in bf_cols), rows, C)
            a = np.zeros((128, C), np.float32)
            a[:rows] = arr2d
            bf_cols.append(a)

        for j in range(k):
            add_bf(f'cur1_{j}', self.cur1_lhsT[j])
        for i in range(NT_W2):
            for j in range(k):
                add_bf(f'w2_{i}_{j}', self.W2_t[j][i])
        for i in range(NT_R1):
            for j in range(k):
                add_bf(f'd1_{i}_{j}', self.d1_t[j][i])
        for i in range(NT_R2):
            for j in range(k):
                add_bf(f'd2_{i}_{j}', self.d2_t[j][i])
        for j in range(k):
            add_bf(f'bias2_{j}', self.bias2_lhsT[j])
        for i in range(NT_ZW):
            add_bf(f'z_{i}', self.Z_t[i])
        bf16 = np.concatenate(bf_cols, axis=1).astype(ml_dtypes.bfloat16)

        f32_cols, off_f32 = [], {}

        def add_f32(name, arr2d):
            rows, C = arr2d.shape
            off_f32[name] = (sum(c.shape[1] for c in f32_cols), rows, C)
            a = np.zeros((128, C), np.float32)
            a[:rows] = arr2d
            f32_cols.append(a)

        add_f32('c1', self.c1)
        add_f32('c2', self.c2)
        add_f32('r1', self.r1)
        add_f32('r2', self.r2)
        add_f32('ld', self.LD)
        for d, M in enumerate(self.LF):
            if M is not None:
                add_f32(f'lf_{d + 1}', M)
        add_f32('lbias', self.l_bias.T.astype(np.float32))
        f32 = np.concatenate(f32_cols, axis=1).astype(np.float32)
        return bf16, f32, off_bf, off_f32


def stage_x(x_core):
    """x_core [T, B] f32 -> [NXROW, T*B] bf16 per XSTAGE_ROWS."""
    import ml_dtypes
    flat = x_core.reshape(-1).astype(np.float32)
    hi = flat.astype(ml_dtypes.bfloat16).astype(np.float32)
    r = flat - hi
    mid = r.astype(ml_dtypes.bfloat16).astype(np.float32)
    lo = (r - mid).astype(np.float32)
    ones = np.ones_like(flat)
    rows = [hi, mid, lo, ones]
    return np.stack([rows[i] for i in XSTAGE_ROWS]).astype(ml_dtypes.bfloat16)


def build_program(T, B_core, off_bf, off_f32, n_bf, n_f32, lf_present, CB, nblk):
    nc = bass.Bass(trn_type="TRN2", target_bir_lowering=False, debug=False,
                   num_devices=NCORES)
    k = K_BLK
    CHUNK = min(128, T)
    nchunk = T // CHUNK

    pbf_d = nc.dram_tensor("pbf", [128, n_bf], BF16, kind="ExternalInput").ap()
    pf_d = nc.dram_tensor("pf", [128, n_f32], F32, kind="ExternalInput").ap()
    xs_d = nc.dram_tensor("xs", [NXROW, T * B_core], BF16,
                          kind="ExternalInput").ap()
    y_d = nc.dram_tensor("y", [T, B_core], F32, kind="ExternalOutput").ap()

    with tile.TileContext(nc) as tc, ExitStack() as ctx:
        const = ctx.enter_context(tc.tile_pool(name="const", bufs=1))
        xpool = ctx.enter_context(tc.tile_pool(name="xpool", bufs=2))
        gpool = ctx.enter_context(tc.tile_pool(name="gpool", bufs=1))
        cpool = ctx.enter_context(tc.tile_pool(name="cpool", bufs=1))
        ypool = ctx.enter_context(tc.tile_pool(name="ypool", bufs=2))
        ps = ctx.enter_context(tc.tile_pool(name="ps", bufs=1, space="PSUM"))
        psL = ctx.enter_context(tc.tile_pool(name="psL", bufs=2, space="PSUM"))

        pbf = const.tile([128, n_bf], BF16)
        pf = const.tile([128, n_f32], F32)
        nc.sync.dma_start(pbf[:], pbf_d[:])
        nc.sync.dma_start(pf[:], pf_d[:])

        def bfp(name):
            o, rows, C = off_bf[name]
            return pbf[0:rows, o:o + C]

        def fpv(name, col, rows=128):
            o, _r, C = off_f32[name]
            return pf[0:rows, o + col:o + col + 1]

        def fpm(name):
            o, rows, C = off_f32[name]
            return pf[0:rows, o:o + C]

        P1 = ps.tile([128, B_core], F32, tag="P1")
        P2 = ps.tile([128, B_core], F32, tag="P2")
        Cb = ps.tile([128, B_core], F32, tag="Cb")

        g1 = [gpool.tile([128, B_core], BF16, tag=f"g1_{i}", name=f"g1_{i}")
              for i in range(2)]
        g2 = [gpool.tile([128, B_core], BF16, tag=f"g2_{i}", name=f"g2_{i}")
              for i in range(2)]
        nc.vector.memset(g1[1][:], 0.0)
        nc.vector.memset(g2[1][:], 0.0)
        ones3 = gpool.tile([3, B_core], BF16, tag="ones3", name="ones3")
        nc.vector.memset(ones3[:], 1.0)

        C_sb = [cpool.tile([128, B_core], F32, tag=f"csb_{i}", name=f"csb_{i}")
                for i in range(nblk)]

        xts = [xpool.tile([NXROW, CHUNK * B_core], BF16, tag="xt",
                          name=f"xt_{i}") for i in range(2)]
        nc.sync.dma_start(xts[0][:], xs_d[:, 0:CHUNK * B_core])
        if nchunk > 1:
            nc.sync.dma_start(xts[1][:],
                              xs_d[:, CHUNK * B_core:2 * CHUNK * B_core])

        for t in range(T):
            j = t % k
            w = t % CB
            blk = t // CB
            cur = t % 2
            prv = 1 - cur
            xt = xts[(t // CHUNK) % 2]
            xo = (t % CHUNK) * B_core

            # ---- layer 1 ----
            nc.tensor.matmul(P1[:], bfp(f'cur1_{j}'),
                             xt[0:NXROW, xo:xo + B_core],
                             start=(t == 0), stop=False, skip_group_check=True)
            for i in range(NT_R1):
                nc.tensor.matmul(P1[:], bfp(f'd1_{i}_{j}'), g1[prv][:],
                                 start=False, stop=False, skip_group_check=True)
            nc.vector.tensor_scalar(g1[cur][:], P1[:], fpv('c1', j), None,
                                    op0=AOP.is_gt)

            # ---- layer 2 ----
            for i in range(NT_W2):
                nc.tensor.matmul(P2[:], bfp(f'w2_{i}_{j}'), g1[cur][:],
                                 start=(t == 0 and i == 0), stop=False,
                                 skip_group_check=True)
            for i in range(NT_R2):
                nc.tensor.matmul(P2[:], bfp(f'd2_{i}_{j}'), g2[prv][:],
                                 start=False, stop=False, skip_group_check=True)
            nc.tensor.matmul(P2[:], bfp(f'bias2_{j}'), ones3[:],
                             start=False, stop=False, skip_group_check=True)
            nc.vector.tensor_scalar(g2[cur][:], P2[:], fpv('c2', j), None,
                                    op0=AOP.is_gt)

            # ---- layer 3 collection ----
            for i in range(NT_ZW):
                nc.tensor.matmul(Cb[0:CB, :],
                                 bfp(f'z_{i}')[:, CB - 1 - w:2 * CB - 1 - w],
                                 g2[cur][:], start=(w == 0 and i == 0),
                                 stop=False, skip_group_check=True)

            # ---- block rescale ----
            if j == k - 1 and t != T - 1:
                nc.vector.tensor_scalar(P1[:], P1[:], fpv('r1', 0), None,
                                        op0=AOP.mult)
                nc.vector.tensor_scalar(P2[:], P2[:], fpv('r2', 0), None,
                                        op0=AOP.mult)

            # ---- C copy out / x prefetch ----
            if w == CB - 1:
                nc.scalar.copy(C_sb[blk][:], Cb[:])
                nxt = t // CHUNK + 2
                if nxt < nchunk:
                    nc.sync.dma_start(
                        xts[nxt % 2][:],
                        xs_d[:, nxt * CHUNK * B_core:(nxt + 1) * CHUNK * B_core])

        # ---- L-scan phase ----
        for i in range(nblk):
            pl = psL.tile([128, B_core], F32, tag="pl", name=f"pl_{i}")
            nc.tensor.matmul(pl[0:CB, :], fpm('ld'), C_sb[i][0:CB, :],
                             start=True, stop=False, skip_group_check=True)
            for d in range(1, i + 1):
                if lf_present[d - 1]:
                    nc.tensor.matmul(pl[0:CB, :], fpm(f'lf_{d}'),
                                     C_sb[i - d][0:CB, :], start=False,
                                     stop=False, skip_group_check=True)
            ysb = ypool.tile([128, B_core], F32, tag="ysb", name=f"ysb_{i}")
            nc.scalar.activation(ysb[0:CB, :], pl[0:CB, :], AFT.Identity,
                                 bias=fpv('lbias', i, CB), scale=1.0)
            nc.sync.dma_start(y_d[CB * i:CB * (i + 1), :], ysb[0:CB, :])

    from waitfix import fix_sync_overflow
    fix_sync_overflow(nc)
    return nc


_PROGRAM_CACHE = {}


def kernel(**inputs):
    x = np.asarray(inputs['x'], np.float32)
    T, B = x.shape[0], x.shape[1]
    B_core = B // NCORES
    x2 = x.reshape(T, B)

    prep = Prep(np.asarray(inputs['W_in'], np.float32),
                np.asarray(inputs['b_in'], np.float32),
                np.asarray(inputs['beta_in'], np.float32),
                np.asarray(inputs['thr_in'], np.float32),
                np.asarray(inputs['W_h'], np.float32),
                np.asarray(inputs['b_h'], np.float32),
                np.asarray(inputs['beta_h'], np.float32),
                np.asarray(inputs['thr_h'], np.float32),
                np.asarray(inputs['W_out'], np.float32),
                np.asarray(inputs['b_out'], np.float32),
                np.asarray(inputs['beta_out'], np.float32), T)
    pbf, pf, off_bf, off_f32 = prep.pack_params()
    lf_present = [M is not None for M in prep.LF]

    key = (T, B_core, pbf.shape[1], pf.shape[1], tuple(lf_present))
    if key not in _PROGRAM_CACHE:
        _PROGRAM_CACHE[key] = build_program(
            T, B_core, off_bf, off_f32, pbf.shape[1], pf.shape[1], lf_present,
            prep.CB, prep.nblk)
    nc = _PROGRAM_CACHE[key]

    in_maps = []
    for c in range(NCORES):
        xc = x2[:, c * B_core:(c + 1) * B_core]
        in_maps.append({'pbf': pbf, 'pf': pf, 'xs': stage_x(xc)})

    res = bass_utils.run_bass_kernel_spmd(nc, in_maps,
                                          core_ids=list(range(NCORES)))
    y = np.concatenate([res.results[c]['y'] for c in range(NCORES)], axis=1)
    return y.reshape(T, B, 1).astype(np.float32)


# revision 7
# speedup vs baseline: 2.4027x; 2.4027x over previous
"""Trainium2 Bass kernel for the 3-layer LIF spiking net (nn_Net_70927089926628).

Reference semantics per timestep t:
    cur1 = x_t * W_in.T + b_in            [B,H]
    m1   = b1*m1 + cur1 - thr1*s1_prev    (reset mask == previous spike)
    s1   = (m1 > thr1)
    cur2 = s1 @ W_h.T + b_h
    m2   = b2*m2 + cur2 - thr2*s2_prev
    s2   = (m2 > thr2)
    m3   = b3*m3 + s2 @ W_out.T + b_out   -> recorded every step (the output)

Mapping:
  - batch 2048 sharded 8 ways -> B=256 per core; params replicated.
  - state feature-major [H=128 partitions, B=256 free].
  - membranes live in PSUM in b^-j scaled form: P_j = b^-j * m_(t0+j) within a
    block of K_BLK steps; every matmul contribution at local step j is
    prescaled by b^-j (folded into host-precomputed stationary operands), so
    the per-step decay multiply disappears; one per-partition rescale op per
    block renormalizes (P *= b^K_BLK).
  - all stationary operands are split into bf16 terms (3 terms ~ exact fp32);
    moving operands are spikes {0,1} (exact in bf16) or split x rows.
  - spikes via DVE tensor_scalar is_gt against c_j = thr * b^-j.
  - layer-3: sliding-window matmul collects cur3 rows into a PSUM tile
    (partition = timestep mod 128); final scan m3 = L @ C as blocked
    lower-triangular fp32 matmuls, + closed-form b_out bias.
"""
import sys
import numpy as np
from contextlib import ExitStack

sys.path.insert(0, '/opt/trn_rl_repo')
if '/root/problem' not in sys.path:
    sys.path.insert(0, '/root/problem')

import concourse.bass as bass
import concourse.tile as tile
import concourse.mybir as mybir
from concourse import bass_utils

F32 = mybir.dt.float32
BF16 = mybir.dt.bfloat16
AOP = mybir.AluOpType
AFT = mybir.ActivationFunctionType

H = 128
NCORES = 8
K_BLK = 8                          # rescale block
B_CLAMP = 1e-5                     # lower clamp on decay factors

# number of bf16 split terms per path (3 ~= exact fp32)
NT_W2 = 3
NT_R1 = 3
NT_R2 = 3
NT_ZW = 2

# cur1 combo: (A-term, x-row) pairs; x rows 0=xhi 1=xmid 2=xlo.
# Bias constants are folded into thresholds + rescale bias (no ones rows).
CUR1_PAIRS = [(0, 0), (0, 1), (1, 0), (0, 2), (1, 1), (2, 0)]
XSTAGE_ROWS = [0, 1, 0, 2, 1, 0]            # source x split row per staged row
NXROW = len(XSTAGE_ROWS)                    # 6


def _split_bf16(a, nterms):
    import ml_dtypes
    out = []
    r = np.asarray(a, np.float32)
    for _ in range(nterms):
        hi = r.astype(ml_dtypes.bfloat16)
        out.append(hi.astype(np.float32))
        r = (r - out[-1]).astype(np.float32)
    return out


def _geom_bias(b_out, b3, T):
    t = np.arange(1, T + 1, dtype=np.float64)
    if abs(1.0 - b3) < 1e-12:
        s = t.astype(np.float64)
    else:
        s = (1.0 - b3 ** t) / (1.0 - b3)
    return (b_out * s).astype(np.float32)


class Prep:
    """Host-side precomputation (shared by all cores)."""

    def __init__(self, W_in, b_in, beta_in, thr_in, W_h, b_h, beta_h, thr_h,
                 W_out, b_out, beta_out, T):
        f64 = np.float64
        self.T = T
        self.CB = min(128, T)              # layer-3 collection block
        self.nblk = (T + self.CB - 1) // self.CB
        k = K_BLK
        b1 = np.clip(beta_in.astype(f64), B_CLAMP, 1.0)
        b2 = np.clip(beta_h.astype(f64), B_CLAMP, 1.0)
        b3 = float(np.clip(beta_out.astype(f64), 0.0, 1.0)[0])
        thr1 = thr_in.astype(f64)
        thr2 = thr_h.astype(f64)
        win = W_in[:, 0].astype(f64)
        wout = W_out[0, :].astype(f64)

        s1 = np.stack([b1 ** (-j) for j in range(k)])      # [k,H]
        s2 = np.stack([b2 ** (-j) for j in range(k)])

        # cur1 combo lhsT_j [NXROW, H]
        self.cur1_lhsT = np.zeros((k, NXROW, H), np.float32)
        for j in range(k):
            A_t = _split_bf16((win * s1[j]).astype(np.float32), 3)
            for r, (a, xi) in enumerate(CUR1_PAIRS):
                self.cur1_lhsT[j, r] = A_t[a]

        # L1 spikes are +/-1 (ACT Sign): s1 = (g1+1)/2. W2 and reset1 operate
        # on g1 with halved coefficients; their constant halves are folded
        # into thresholds / rescale bias below.
        # W2_j: lhsT[k=h1, m=h2] = W_h[h2,h1]/2 * s2_j[h2]
        W2 = W_h.astype(f64).T[None, :, :] * s2[:, None, :] * 0.5
        self.W2_t = []
        for j in range(k):
            self.W2_t.append(_split_bf16(W2[j].astype(np.float32), NT_W2))
        # reset diags: L1 halved (g1 in +/-1), L2 plain (g2 in {0,1})
        self.d1_t, self.d2_t = [], []
        for j in range(k):
            self.d1_t.append([np.diag(v) for v in _split_bf16(
                (-(thr1 * s1[j]) * 0.5).astype(np.float32), NT_R1)])
            self.d2_t.append([np.diag(v) for v in _split_bf16(
                (-(thr2 * s2[j])).astype(np.float32), NT_R2)])

        # constant per-step inflows (folded, not matmul'd):
        beta1 = b_in.astype(f64) - 0.5 * thr1                 # [H]
        beta2 = b_h.astype(f64) + 0.5 * W_h.astype(f64).sum(axis=1)
        # D_j = sum_{i<=j} b^-i * beta  (missing accumulated bias at local j)
        D1 = np.cumsum(s1 * beta1[None, :], axis=0)           # [k,H]
        D2 = np.cumsum(s2 * beta2[None, :], axis=0)
        # effective thresholds c'_j = thr*b^-j - D_j
        c1p = thr1[None, :] * s1 - D1
        c2p = thr2[None, :] * s2 - D2
        self.c1n = (-c1p).astype(np.float32).T                # [H,k] Sign bias
        self.c2 = c2p.astype(np.float32).T                    # [H,k]
        self.r1 = (b1 ** k).astype(np.float32)[:, None]       # [H,1]
        self.r2 = (b2 ** k).astype(np.float32)[:, None]
        # rescale bias: restore the bias sum at block end
        self.rb1 = ((b1 ** k) * D1[k - 1]).astype(np.float32)[:, None]
        self.rb2 = ((b2 ** k) * D2[k - 1]).astype(np.float32)[:, None]

        # layer-3 Z buffers [H, 2*CB-1] with W_out term at col CB-1
        CB = self.CB
        wout_t = _split_bf16(wout.astype(np.float32), NT_ZW)
        self.Z_t = []
        for i in range(NT_ZW):
            Z = np.zeros((H, 2 * CB - 1), np.float32)
            Z[:, CB - 1] = wout_t[i]
            self.Z_t.append(Z)

        # L-scan matrices (fp32) [CB, CB]
        idx = np.arange(CB)
        dt_ = idx[None, :] - idx[:, None]                     # t - tau
        with np.errstate(over='ignore', under='ignore'):
            LD = np.where(dt_ >= 0, b3 ** np.maximum(dt_, 0), 0.0)
        self.LD = LD.astype(np.float32)
        self.LF = []
        for d in range(1, self.nblk):
            with np.errstate(over='ignore', under='ignore'):
                M = b3 ** (dt_.astype(f64) + CB * d)
            M = np.where(np.isfinite(M), M, 0.0).astype(np.float32)
            self.LF.append(None if np.abs(M).max() < 1e-37 else M)
        self.l_bias = _geom_bias(float(np.asarray(b_out).ravel()[0]), b3,
                                 T).reshape(self.nblk, self.CB)

    def pack_params(self):
        import ml_dtypes
        k = K_BLK
        bf_cols, off_bf = [], {}

        def add_bf(name, arr2d):
            rows, C = arr2d.shape
            off_bf[name] = (sum(c.shape[1] for c in bf_cols), rows, C)
            a = np.zeros((128, C), np.float32)
            a[:rows] = arr2d
            bf_cols.append(a)

        for j in range(k):
            add_bf(f'cur1_{j}', self.cur1_lhsT[j])
        for i in range(NT_W2):
            for j in range(k):
                add_bf(f'w2_{i}_{j}', self.W2_t[j][i])
        for i in range(NT_R1):
            for j in range(k):
                add_bf(f'd1_{i}_{j}', self.d1_t[j][i])
        for i in range(NT_R2):
            for j in range(k):
                add_bf(f'd2_{i}_{j}', self.d2_t[j][i])
        for i in range(NT_ZW):
            add_bf(f'z_{i}', self.Z_t[i])
        bf16 = np.concatenate(bf_cols, axis=1).astype(ml_dtypes.bfloat16)

        f32_cols, off_f32 = [], {}

        def add_f32(name, arr2d):
            rows, C = arr2d.shape
            off_f32[name] = (sum(c.shape[1] for c in f32_cols), rows, C)
            a = np.zeros((128, C), np.float32)
            a[:rows] = arr2d
            f32_cols.append(a)

        add_f32('c1n', self.c1n)
        add_f32('c2', self.c2)
        add_f32('r1', self.r1)
        add_f32('r2', self.r2)
        add_f32('rb1', self.rb1)
        add_f32('rb2', self.rb2)
        add_f32('ld', self.LD)
        for d, M in enumerate(self.LF):
            if M is not None:
                add_f32(f'lf_{d + 1}', M)
        add_f32('lbias', self.l_bias.T.astype(np.float32))
        f32 = np.concatenate(f32_cols, axis=1).astype(np.float32)
        return bf16, f32, off_bf, off_f32


def stage_x(x_core):
    """x_core [T, B] f32 -> [NXROW, T*B] bf16 per XSTAGE_ROWS."""
    import ml_dtypes
    flat = x_core.reshape(-1).astype(np.float32)
    hi = flat.astype(ml_dtypes.bfloat16).astype(np.float32)
    r = flat - hi
    mid = r.astype(ml_dtypes.bfloat16).astype(np.float32)
    lo = (r - mid).astype(np.float32)
    rows = [hi, mid, lo]
    return np.stack([rows[i] for i in XSTAGE_ROWS]).astype(ml_dtypes.bfloat16)


def build_program(T, B_core, off_bf, off_f32, n_bf, n_f32, lf_present, CB, nblk):
    nc = bass.Bass(trn_type="TRN2", target_bir_lowering=False, debug=False,
                   num_devices=NCORES)
    k = K_BLK
    CHUNK = min(128, T)
    nchunk = T // CHUNK

    pbf_d = nc.dram_tensor("pbf", [128, n_bf], BF16, kind="ExternalInput").ap()
    pf_d = nc.dram_tensor("pf", [128, n_f32], F32, kind="ExternalInput").ap()
    xs_d = nc.dram_tensor("xs", [NXROW, T * B_core], BF16,
                          kind="ExternalInput").ap()
    y_d = nc.dram_tensor("y", [T, B_core], F32, kind="ExternalOutput").ap()

    with tile.TileContext(nc) as tc, ExitStack() as ctx:
        const = ctx.enter_context(tc.tile_pool(name="const", bufs=1))
        xpool = ctx.enter_context(tc.tile_pool(name="xpool", bufs=2))
        gpool = ctx.enter_context(tc.tile_pool(name="gpool", bufs=1))
        cpool = ctx.enter_context(tc.tile_pool(name="cpool", bufs=1))
        ypool = ctx.enter_context(tc.tile_pool(name="ypool", bufs=2))
        ps = ctx.enter_context(tc.tile_pool(name="ps", bufs=1, space="PSUM"))
        psL = ctx.enter_context(tc.tile_pool(name="psL", bufs=2, space="PSUM"))

        pbf = const.tile([128, n_bf], BF16)
        pf = const.tile([128, n_f32], F32)
        nc.sync.dma_start(pbf[:], pbf_d[:])
        nc.sync.dma_start(pf[:], pf_d[:])

        def bfp(name):
            o, rows, C = off_bf[name]
            return pbf[0:rows, o:o + C]

        def fpv(name, col, rows=128):
            o, _r, C = off_f32[name]
            return pf[0:rows, o + col:o + col + 1]

        def fpm(name):
            o, rows, C = off_f32[name]
            return pf[0:rows, o:o + C]

        P1 = ps.tile([128, B_core], F32, tag="P1")
        P2 = ps.tile([128, B_core], F32, tag="P2")
        Cb = ps.tile([128, B_core], F32, tag="Cb")

        g1 = [gpool.tile([128, B_core], BF16, tag=f"g1_{i}", name=f"g1_{i}")
              for i in range(2)]
        g2 = [gpool.tile([128, B_core], BF16, tag=f"g2_{i}", name=f"g2_{i}")
              for i in range(2)]
        nc.vector.memset(g1[1][:], -1.0)   # s1_prev=0 in +/-1 encoding
        nc.vector.memset(g2[1][:], 0.0)

        C_sb = [cpool.tile([128, B_core], F32, tag=f"csb_{i}", name=f"csb_{i}")
                for i in range(nblk)]

        xts = [xpool.tile([NXROW, CHUNK * B_core], BF16, tag="xt",
                          name=f"xt_{i}") for i in range(2)]
        nc.sync.dma_start(xts[0][:], xs_d[:, 0:CHUNK * B_core])
        if nchunk > 1:
            nc.sync.dma_start(xts[1][:],
                              xs_d[:, CHUNK * B_core:2 * CHUNK * B_core])

        for t in range(T):
            j = t % k
            w = t % CB
            blk = t // CB
            cur = t % 2
            prv = 1 - cur
            xt = xts[(t // CHUNK) % 2]
            xo = (t % CHUNK) * B_core

            # ---- layer 1 ----
            nc.tensor.matmul(P1[:], bfp(f'cur1_{j}'),
                             xt[0:NXROW, xo:xo + B_core],
                             start=(t == 0), stop=False, skip_group_check=True)
            for i in range(NT_R1):
                nc.tensor.matmul(P1[:], bfp(f'd1_{i}_{j}'), g1[prv][:],
                                 start=False, stop=False, skip_group_check=True)
            nc.scalar.activation(g1[cur][:], P1[:], AFT.Sign,
                                 bias=fpv('c1n', j), scale=1.0)

            # ---- layer 2 ----
            for i in range(NT_W2):
                nc.tensor.matmul(P2[:], bfp(f'w2_{i}_{j}'), g1[cur][:],
                                 start=(t == 0 and i == 0), stop=False,
                                 skip_group_check=True)
            for i in range(NT_R2):
                nc.tensor.matmul(P2[:], bfp(f'd2_{i}_{j}'), g2[prv][:],
                                 start=False, stop=False, skip_group_check=True)
            nc.vector.tensor_scalar(g2[cur][:], P2[:], fpv('c2', j), None,
                                    op0=AOP.is_gt)

            # ---- layer 3 collection ----
            for i in range(NT_ZW):
                nc.tensor.matmul(Cb[0:CB, :],
                                 bfp(f'z_{i}')[:, CB - 1 - w:2 * CB - 1 - w],
                                 g2[cur][:], start=(w == 0 and i == 0),
                                 stop=False, skip_group_check=True)

            # ---- block rescale (+ bias restore) ----
            if j == k - 1 and t != T - 1:
                nc.scalar.activation(P1[:], P1[:], AFT.Identity,
                                     bias=fpv('rb1', 0), scale=fpv('r1', 0))
                nc.scalar.activation(P2[:], P2[:], AFT.Identity,
                                     bias=fpv('rb2', 0), scale=fpv('r2', 0))

            # ---- C copy out / x prefetch ----
            if w == CB - 1:
                nc.scalar.copy(C_sb[blk][:], Cb[:])
                nxt = t // CHUNK + 2
                if nxt < nchunk:
                    nc.sync.dma_start(
                        xts[nxt % 2][:],
                        xs_d[:, nxt * CHUNK * B_core:(nxt + 1) * CHUNK * B_core])

        # ---- L-scan phase ----
        for i in range(nblk):
            pl = psL.tile([128, B_core], F32, tag="pl", name=f"pl_{i}")
            nc.tensor.matmul(pl[0:CB, :], fpm('ld'), C_sb[i][0:CB, :],
                             start=True, stop=False, skip_group_check=True)
            for d in range(1, i + 1):
                if lf_present[d - 1]:
                    nc.tensor.matmul(pl[0:CB, :], fpm(f'lf_{d}'),
                                     C_sb[i - d][0:CB, :], start=False,
                                     stop=False, skip_group_check=True)
            ysb = ypool.tile([128, B_core], F32, tag="ysb", name=f"ysb_{i}")
            nc.scalar.activation(ysb[0:CB, :], pl[0:CB, :], AFT.Identity,
                                 bias=fpv('lbias', i, CB), scale=1.0)
            nc.sync.dma_start(y_d[CB * i:CB * (i + 1), :], ysb[0:CB, :])

    from waitfix import fix_sync_overflow
    fix_sync_overflow(nc)
    return nc


_PROGRAM_CACHE = {}


def kernel(**inputs):
    x = np.asarray(inputs['x'], np.float32)
    T, B = x.shape[0], x.shape[1]
    B_core = B // NCORES
    x2 = x.reshape(T, B)

    prep = Prep(np.asarray(inputs['W_in'], np.float32),
                np.asarray(inputs['b_in'], np.float32),
                np.asarray(inputs['beta_in'], np.float32),
                np.asarray(inputs['thr_in'], np.float32),
                np.asarray(inputs['W_h'], np.float32),
                np.asarray(inputs['b_h'], np.float32),
                np.asarray(inputs['beta_h'], np.float32),
                np.asarray(inputs['thr_h'], np.float32),
                np.asarray(inputs['W_out'], np.float32),
                np.asarray(inputs['b_out'], np.float32),
                np.asarray(inputs['beta_out'], np.float32), T)
    pbf, pf, off_bf, off_f32 = prep.pack_params()
    lf_present = [M is not None for M in prep.LF]

    key = (T, B_core, pbf.shape[1], pf.shape[1], tuple(lf_present))
    if key not in _PROGRAM_CACHE:
        _PROGRAM_CACHE[key] = build_program(
            T, B_core, off_bf, off_f32, pbf.shape[1], pf.shape[1], lf_present,
            prep.CB, prep.nblk)
    nc = _PROGRAM_CACHE[key]

    in_maps = []
    for c in range(NCORES):
        xc = x2[:, c * B_core:(c + 1) * B_core]
        in_maps.append({'pbf': pbf, 'pf': pf, 'xs': stage_x(xc)})

    res = bass_utils.run_bass_kernel_spmd(nc, in_maps,
                                          core_ids=list(range(NCORES)))
    y = np.concatenate([res.results[c]['y'] for c in range(NCORES)], axis=1)
    return y.reshape(T, B, 1).astype(np.float32)


# revision 9
# speedup vs baseline: 2.4862x; 1.0348x over previous
"""Trainium2 Bass kernel for the 3-layer LIF spiking net (nn_Net_70927089926628).

Reference semantics per timestep t:
    cur1 = x_t * W_in.T + b_in            [B,H]
    m1   = b1*m1 + cur1 - thr1*s1_prev    (reset mask == previous spike)
    s1   = (m1 > thr1)
    cur2 = s1 @ W_h.T + b_h
    m2   = b2*m2 + cur2 - thr2*s2_prev
    s2   = (m2 > thr2)
    m3   = b3*m3 + s2 @ W_out.T + b_out   -> recorded every step (the output)

Mapping:
  - batch 2048 sharded 8 ways -> B=256 per core; params replicated.
  - state feature-major [H=128 partitions, B=256 free].
  - membranes live in PSUM in b^-j scaled form: P_j = b^-j * m_(t0+j) within a
    block of K_BLK steps; every matmul contribution at local step j is
    prescaled by b^-j (folded into host-precomputed stationary operands), so
    the per-step decay multiply disappears; one per-partition rescale op per
    block renormalizes (P *= b^K_BLK).
  - all stationary operands are split into bf16 terms (3 terms ~ exact fp32);
    moving operands are spikes {0,1} (exact in bf16) or split x rows.
  - spikes via DVE tensor_scalar is_gt against c_j = thr * b^-j.
  - layer-3: sliding-window matmul collects cur3 rows into a PSUM tile
    (partition = timestep mod 128); final scan m3 = L @ C as blocked
    lower-triangular fp32 matmuls, + closed-form b_out bias.
"""
import sys
import numpy as np
from contextlib import ExitStack

sys.path.insert(0, '/opt/trn_rl_repo')
if '/root/problem' not in sys.path:
    sys.path.insert(0, '/root/problem')

import concourse.bass as bass
import concourse.tile as tile
import concourse.mybir as mybir
from concourse import bass_utils

F32 = mybir.dt.float32
BF16 = mybir.dt.bfloat16
AOP = mybir.AluOpType
AFT = mybir.ActivationFunctionType

H = 128
NCORES = 8
K_BLK = 8                          # rescale block
B_CLAMP = 1e-5                     # lower clamp on decay factors

# number of bf16 split terms per path (3 ~= exact fp32)
NT_W2 = 2
NT_R1 = 3
NT_R2 = 2
NT_ZW = 2

# cur1 combo: (A-term, x-row) pairs; x rows 0=xhi 1=xmid 2=xlo.
# Bias constants are folded into thresholds + rescale bias (no ones rows).
CUR1_PAIRS = [(0, 0), (0, 1), (1, 0), (0, 2), (1, 1), (2, 0)]
XSTAGE_ROWS = [0, 1, 0, 2, 1, 0]            # source x split row per staged row
NXROW = len(XSTAGE_ROWS)                    # 6


def _split_bf16(a, nterms):
    import ml_dtypes
    out = []
    r = np.asarray(a, np.float32)
    for _ in range(nterms):
        hi = r.astype(ml_dtypes.bfloat16)
        out.append(hi.astype(np.float32))
        r = (r - out[-1]).astype(np.float32)
    return out


def _geom_bias(b_out, b3, T):
    t = np.arange(1, T + 1, dtype=np.float64)
    if abs(1.0 - b3) < 1e-12:
        s = t.astype(np.float64)
    else:
        s = (1.0 - b3 ** t) / (1.0 - b3)
    return (b_out * s).astype(np.float32)


class Prep:
    """Host-side precomputation (shared by all cores)."""

    def __init__(self, W_in, b_in, beta_in, thr_in, W_h, b_h, beta_h, thr_h,
                 W_out, b_out, beta_out, T):
        f64 = np.float64
        self.T = T
        self.CB = min(128, T)              # layer-3 collection block
        self.nblk = (T + self.CB - 1) // self.CB
        k = K_BLK
        b1 = np.clip(beta_in.astype(f64), B_CLAMP, 1.0)
        b2 = np.clip(beta_h.astype(f64), B_CLAMP, 1.0)
        b3 = float(np.clip(beta_out.astype(f64), 0.0, 1.0)[0])
        thr1 = thr_in.astype(f64)
        thr2 = thr_h.astype(f64)
        win = W_in[:, 0].astype(f64)
        wout = W_out[0, :].astype(f64)

        s1 = np.stack([b1 ** (-j) for j in range(k)])      # [k,H]
        s2 = np.stack([b2 ** (-j) for j in range(k)])

        # cur1 combo lhsT_j [NXROW, H]
        self.cur1_lhsT = np.zeros((k, NXROW, H), np.float32)
        for j in range(k):
            A_t = _split_bf16((win * s1[j]).astype(np.float32), 3)
            for r, (a, xi) in enumerate(CUR1_PAIRS):
                self.cur1_lhsT[j, r] = A_t[a]

        # L1 spikes are +/-1 (ACT Sign): s1 = (g1+1)/2. W2 and reset1 operate
        # on g1 with halved coefficients; their constant halves are folded
        # into thresholds / rescale bias below.
        # W2_j: lhsT[k=h1, m=h2] = W_h[h2,h1]/2 * s2_j[h2]
        W2 = W_h.astype(f64).T[None, :, :] * s2[:, None, :] * 0.5
        self.W2_t = []
        for j in range(k):
            self.W2_t.append(_split_bf16(W2[j].astype(np.float32), NT_W2))
        # reset diags: L1 halved (g1 in +/-1), L2 plain (g2 in {0,1})
        self.d1_t, self.d2_t = [], []
        for j in range(k):
            self.d1_t.append([np.diag(v) for v in _split_bf16(
                (-(thr1 * s1[j]) * 0.5).astype(np.float32), NT_R1)])
            self.d2_t.append([np.diag(v) for v in _split_bf16(
                (-(thr2 * s2[j])).astype(np.float32), NT_R2)])

        # constant per-step inflows (folded, not matmul'd):
        beta1 = b_in.astype(f64) - 0.5 * thr1                 # [H]
        beta2 = b_h.astype(f64) + 0.5 * W_h.astype(f64).sum(axis=1)
        # D_j = sum_{i<=j} b^-i * beta  (missing accumulated bias at local j)
        D1 = np.cumsum(s1 * beta1[None, :], axis=0)           # [k,H]
        D2 = np.cumsum(s2 * beta2[None, :], axis=0)
        # effective thresholds c'_j = thr*b^-j - D_j
        c1p = thr1[None, :] * s1 - D1
        c2p = thr2[None, :] * s2 - D2
        self.c1n = (-c1p).astype(np.float32).T                # [H,k] Sign bias
        self.c2 = c2p.astype(np.float32).T                    # [H,k]
        self.r1 = (b1 ** k).astype(np.float32)[:, None]       # [H,1]
        self.r2 = (b2 ** k).astype(np.float32)[:, None]
        # rescale bias: restore the bias sum at block end
        self.rb1 = ((b1 ** k) * D1[k - 1]).astype(np.float32)[:, None]
        self.rb2 = ((b2 ** k) * D2[k - 1]).astype(np.float32)[:, None]

        # layer-3 Z buffers, even/odd parity so the sliding lhsT slice is
        # always 4-byte aligned in bf16.
        CB = self.CB
        wout_t = _split_bf16(wout.astype(np.float32), NT_ZW)
        self.Z_t, self.Zo_t = [], []
        for i in range(NT_ZW):
            Z = np.zeros((H, 2 * CB), np.float32)
            Z[:, CB - 1] = wout_t[i]
            self.Z_t.append(Z[:, :2 * CB - 1])
            Zo = np.zeros((H, 2 * CB + 1), np.float32)
            Zo[:, CB + 1] = wout_t[i]
            self.Zo_t.append(Zo)

        # L-scan matrices (fp32) [CB, CB]
        idx = np.arange(CB)
        dt_ = idx[None, :] - idx[:, None]                     # t - tau
        with np.errstate(over='ignore', under='ignore'):
            LD = np.where(dt_ >= 0, b3 ** np.maximum(dt_, 0), 0.0)
        self.LD = LD.astype(np.float32)
        self.LF = []
        for d in range(1, self.nblk):
            with np.errstate(over='ignore', under='ignore'):
                M = b3 ** (dt_.astype(f64) + CB * d)
            M = np.where(np.isfinite(M), M, 0.0).astype(np.float32)
            self.LF.append(None if np.abs(M).max() < 1e-37 else M)
        self.l_bias = _geom_bias(float(np.asarray(b_out).ravel()[0]), b3,
                                 T).reshape(self.nblk, self.CB)

    def pack_params(self):
        import ml_dtypes
        k = K_BLK
        bf_cols, off_bf = [], {}

        def add_bf(name, arr2d):
            rows, C = arr2d.shape
            off_bf[name] = (sum(c.shape[1] for c in bf_cols), rows, C)
            a = np.zeros((128, C), np.float32)
            a[:rows] = arr2d
            bf_cols.append(a)

        for j in range(k):
            add_bf(f'cur1_{j}', self.cur1_lhsT[j])
        for i in range(NT_W2):
            for j in range(k):
                add_bf(f'w2_{i}_{j}', self.W2_t[j][i])
        for i in range(NT_R1):
            for j in range(k):
                add_bf(f'd1_{i}_{j}', self.d1_t[j][i])
        for i in range(NT_R2):
            for j in range(k):
                add_bf(f'd2_{i}_{j}', self.d2_t[j][i])
        for i in range(NT_ZW):
            add_bf(f'z_{i}', self.Z_t[i])
            add_bf(f'zo_{i}', self.Zo_t[i])
        bf16 = np.concatenate(bf_cols, axis=1).astype(ml_dtypes.bfloat16)

        f32_cols, off_f32 = [], {}

        def add_f32(name, arr2d):
            rows, C = arr2d.shape
            off_f32[name] = (sum(c.shape[1] for c in f32_cols), rows, C)
            a = np.zeros((128, C), np.float32)
            a[:rows] = arr2d
            f32_cols.append(a)

        add_f32('c1n', self.c1n)
        add_f32('c2', self.c2)
        add_f32('r1', self.r1)
        add_f32('r2', self.r2)
        add_f32('rb1', self.rb1)
        add_f32('rb2', self.rb2)
        add_f32('ld', self.LD)
        for d, M in enumerate(self.LF):
            if M is not None:
                add_f32(f'lf_{d + 1}', M)
        add_f32('lbias', self.l_bias.T.astype(np.float32))
        f32 = np.concatenate(f32_cols, axis=1).astype(np.float32)
        return bf16, f32, off_bf, off_f32


def stage_x(x_core):
    """x_core [T, B] f32 -> [NXROW, T*B] bf16 per XSTAGE_ROWS."""
    import ml_dtypes
    flat = x_core.reshape(-1).astype(np.float32)
    hi = flat.astype(ml_dtypes.bfloat16).astype(np.float32)
    r = flat - hi
    mid = r.astype(ml_dtypes.bfloat16).astype(np.float32)
    lo = (r - mid).astype(np.float32)
    rows = [hi, mid, lo]
    return np.stack([rows[i] for i in XSTAGE_ROWS]).astype(ml_dtypes.bfloat16)


def build_program(T, B_core, off_bf, off_f32, n_bf, n_f32, lf_present, CB, nblk):
    nc = bass.Bass(trn_type="TRN2", target_bir_lowering=False, debug=False,
                   num_devices=NCORES)
    k = K_BLK
    CHUNK = min(128, T)
    nchunk = T // CHUNK

    pbf_d = nc.dram_tensor("pbf", [128, n_bf], BF16, kind="ExternalInput").ap()
    pf_d = nc.dram_tensor("pf", [128, n_f32], F32, kind="ExternalInput").ap()
    xs_d = nc.dram_tensor("xs", [NXROW, T * B_core], BF16,
                          kind="ExternalInput").ap()
    y_d = nc.dram_tensor("y", [T, B_core], F32, kind="ExternalOutput").ap()

    with tile.TileContext(nc) as tc, ExitStack() as ctx:
        const = ctx.enter_context(tc.tile_pool(name="const", bufs=1))
        xpool = ctx.enter_context(tc.tile_pool(name="xpool", bufs=2))
        gpool = ctx.enter_context(tc.tile_pool(name="gpool", bufs=1))
        cpool = ctx.enter_context(tc.tile_pool(name="cpool", bufs=1))
        ypool = ctx.enter_context(tc.tile_pool(name="ypool", bufs=2))
        ps = ctx.enter_context(tc.tile_pool(name="ps", bufs=1, space="PSUM"))
        psL = ctx.enter_context(tc.tile_pool(name="psL", bufs=2, space="PSUM"))

        pbf = const.tile([128, n_bf], BF16)
        pf = const.tile([128, n_f32], F32)
        nc.sync.dma_start(pbf[:], pbf_d[:])
        nc.sync.dma_start(pf[:], pf_d[:])

        def bfp(name):
            o, rows, C = off_bf[name]
            return pbf[0:rows, o:o + C]

        def fpv(name, col, rows=128):
            o, _r, C = off_f32[name]
            return pf[0:rows, o + col:o + col + 1]

        def fpm(name):
            o, rows, C = off_f32[name]
            return pf[0:rows, o:o + C]

        P1 = ps.tile([128, B_core], F32, tag="P1")
        P2 = ps.tile([128, B_core], F32, tag="P2")
        Cb = ps.tile([128, B_core], F32, tag="Cb")

        g1 = [gpool.tile([128, B_core], BF16, tag=f"g1_{i}", name=f"g1_{i}")
              for i in range(2)]
        g2 = [gpool.tile([128, B_core], BF16, tag=f"g2_{i}", name=f"g2_{i}")
              for i in range(2)]
        nc.vector.memset(g1[1][:], -1.0)   # s1_prev=0 in +/-1 encoding
        nc.vector.memset(g2[1][:], 0.0)

        C_sb = [cpool.tile([128, B_core], F32, tag=f"csb_{i}", name=f"csb_{i}")
                for i in range(nblk)]

        xts = [xpool.tile([NXROW, CHUNK * B_core], BF16, tag="xt",
                          name=f"xt_{i}") for i in range(2)]
        nc.sync.dma_start(xts[0][:], xs_d[:, 0:CHUNK * B_core])
        if nchunk > 1:
            nc.sync.dma_start(xts[1][:],
                              xs_d[:, CHUNK * B_core:2 * CHUNK * B_core])

        for t in range(T):
            j = t % k
            w = t % CB
            blk = t // CB
            cur = t % 2
            prv = 1 - cur
            xt = xts[(t // CHUNK) % 2]
            xo = (t % CHUNK) * B_core

            # ---- layer 1 ----
            nc.tensor.matmul(P1[:], bfp(f'cur1_{j}'),
                             xt[0:NXROW, xo:xo + B_core],
                             start=(t == 0), stop=False, skip_group_check=True)
            for i in range(NT_R1):
                nc.tensor.matmul(P1[:], bfp(f'd1_{i}_{j}'), g1[prv][:],
                                 start=False, stop=False, skip_group_check=True)
            nc.scalar.activation(g1[cur][:], P1[:], AFT.Sign,
                                 bias=fpv('c1n', j), scale=1.0)

            # ---- layer 2 ----
            for i in range(NT_W2):
                nc.tensor.matmul(P2[:], bfp(f'w2_{i}_{j}'), g1[cur][:],
                                 start=(t == 0 and i == 0), stop=False,
                                 skip_group_check=True)
            for i in range(NT_R2):
                nc.tensor.matmul(P2[:], bfp(f'd2_{i}_{j}'), g2[prv][:],
                                 start=False, stop=False, skip_group_check=True)
            nc.vector.tensor_scalar(g2[cur][:], P2[:], fpv('c2', j), None,
                                    op0=AOP.is_gt)

            # ---- layer 3 collection (emitted one step late to fill
            #      PE stalls; flush at block end) ----
            def emit_zwin(tz):
                wz = tz % CB
                gz = g2[tz % 2]
                for i in range(NT_ZW):
                    if wz % 2 == 0:
                        lhs = bfp(f'z_{i}')[:, CB - 1 - wz:2 * CB - 1 - wz]
                    else:
                        lhs = bfp(f'zo_{i}')[:, CB + 1 - wz:2 * CB + 1 - wz]
                    nc.tensor.matmul(Cb[0:CB, :], lhs, gz[:],
                                     start=(wz == 0 and i == 0), stop=False,
                                     skip_group_check=True)
            if t > 0 and t % CB != 0:
                emit_zwin(t - 1)
            if w == CB - 1:
                emit_zwin(t)

            # ---- block rescale (+ bias restore): fused mult+add on DVE ----
            if j == k - 1 and t != T - 1:
                nc.vector.tensor_scalar(P1[:], P1[:], fpv('r1', 0),
                                        fpv('rb1', 0), op0=AOP.mult,
                                        op1=AOP.add)
                nc.vector.tensor_scalar(P2[:], P2[:], fpv('r2', 0),
                                        fpv('rb2', 0), op0=AOP.mult,
                                        op1=AOP.add)

            # ---- C copy out / x prefetch ----
            if w == CB - 1:
                nc.scalar.copy(C_sb[blk][:], Cb[:])
                nxt = t // CHUNK + 2
                if nxt < nchunk:
                    nc.sync.dma_start(
                        xts[nxt % 2][:],
                        xs_d[:, nxt * CHUNK * B_core:(nxt + 1) * CHUNK * B_core])

        # ---- L-scan phase ----
        for i in range(nblk):
            pl = psL.tile([128, B_core], F32, tag="pl", name=f"pl_{i}")
            nc.tensor.matmul(pl[0:CB, :], fpm('ld'), C_sb[i][0:CB, :],
                             start=True, stop=False, skip_group_check=True)
            for d in range(1, i + 1):
                if lf_present[d - 1]:
                    nc.tensor.matmul(pl[0:CB, :], fpm(f'lf_{d}'),
                                     C_sb[i - d][0:CB, :], start=False,
                                     stop=False, skip_group_check=True)
            ysb = ypool.tile([128, B_core], F32, tag="ysb", name=f"ysb_{i}")
            nc.scalar.activation(ysb[0:CB, :], pl[0:CB, :], AFT.Identity,
                                 bias=fpv('lbias', i, CB), scale=1.0)
            nc.sync.dma_start(y_d[CB * i:CB * (i + 1), :], ysb[0:CB, :])

    from waitfix import fix_sync_overflow
    fix_sync_overflow(nc)
    return nc


_PROGRAM_CACHE = {}


def kernel(**inputs):
    x = np.asarray(inputs['x'], np.float32)
    T, B = x.shape[0], x.shape[1]
    B_core = B // NCORES
    x2 = x.reshape(T, B)

    prep = Prep(np.asarray(inputs['W_in'], np.float32),
                np.asarray(inputs['b_in'], np.float32),
                np.asarray(inputs['beta_in'], np.float32),
                np.asarray(inputs['thr_in'], np.float32),
                np.asarray(inputs['W_h'], np.float32),
                np.asarray(inputs['b_h'], np.float32),
                np.asarray(inputs['beta_h'], np.float32),
                np.asarray(inputs['thr_h'], np.float32),
                np.asarray(inputs['W_out'], np.float32),
                np.asarray(inputs['b_out'], np.float32),
                np.asarray(inputs['beta_out'], np.float32), T)
    pbf, pf, off_bf, off_f32 = prep.pack_params()
    lf_present = [M is not None for M in prep.LF]

    key = (T, B_core, pbf.shape[1], pf.shape[1], tuple(lf_present))
    if key not in _PROGRAM_CACHE:
        _PROGRAM_CACHE[key] = build_program(
            T, B_core, off_bf, off_f32, pbf.shape[1], pf.shape[1], lf_present,
            prep.CB, prep.nblk)
    nc = _PROGRAM_CACHE[key]

    in_maps = []
    for c in range(NCORES):
        xc = x2[:, c * B_core:(c + 1) * B_core]
        in_maps.append({'pbf': pbf, 'pf': pf, 'xs': stage_x(xc)})

    res = bass_utils.run_bass_kernel_spmd(nc, in_maps,
                                          core_ids=list(range(NCORES)))
    y = np.concatenate([res.results[c]['y'] for c in range(NCORES)], axis=1)
    return y.reshape(T, B, 1).astype(np.float32)


# revision 12
# speedup vs baseline: 1527.0413x; 614.2115x over previous
"""Trainium2 Bass kernel for the 3-layer LIF spiking net (nn_Net_70927089926628).

Reference semantics per timestep t:
    cur1 = x_t * W_in.T + b_in            [B,H]
    m1   = b1*m1 + cur1 - thr1*s1_prev    (reset mask == previous spike)
    s1   = (m1 > thr1)
    cur2 = s1 @ W_h.T + b_h
    m2   = b2*m2 + cur2 - thr2*s2_prev
    s2   = (m2 > thr2)
    m3   = b3*m3 + s2 @ W_out.T + b_out   -> recorded every step (the output)

Mapping:
  - batch 2048 sharded 8 ways -> B=256 per core; params replicated.
  - state feature-major [H=128 partitions, B=256 free].
  - membranes live in PSUM in b^-j scaled form: P_j = b^-j * m_(t0+j) within a
    block of K_BLK steps; every matmul contribution at local step j is
    prescaled by b^-j (folded into host-precomputed stationary operands), so
    the per-step decay multiply disappears; one per-partition rescale op per
    block renormalizes (P *= b^K_BLK).
  - all stationary operands are split into bf16 terms (3 terms ~ exact fp32);
    moving operands are spikes {0,1} (exact in bf16) or split x rows.
  - spikes via DVE tensor_scalar is_gt against c_j = thr * b^-j.
  - layer-3: sliding-window matmul collects cur3 rows into a PSUM tile
    (partition = timestep mod 128); final scan m3 = L @ C as blocked
    lower-triangular fp32 matmuls, + closed-form b_out bias.
"""
import sys
import numpy as np
from contextlib import ExitStack

sys.path.insert(0, '/opt/trn_rl_repo')

import concourse.bass as bass
import concourse.tile as tile
import concourse.mybir as mybir
from concourse import bass_utils

F32 = mybir.dt.float32
BF16 = mybir.dt.bfloat16
AOP = mybir.AluOpType
AFT = mybir.ActivationFunctionType

H = 128
NCORES = 8
K_BLK = 8                          # rescale block
B_CLAMP = 1e-5                     # lower clamp on decay factors

# number of bf16 split terms per path (3 ~= exact fp32)
NT_W2 = 2
NT_R1 = 3
NT_R2 = 2
NT_ZW = 2

# cur1 combo: (A-term, x-row) pairs; x rows 0=xhi 1=xmid 2=xlo.
# Bias constants are folded into thresholds + rescale bias (no ones rows).
CUR1_PAIRS = [(0, 0), (0, 1), (1, 0), (0, 2), (1, 1), (2, 0)]
XSTAGE_ROWS = [0, 1, 0, 2, 1, 0]            # source x split row per staged row
NXROW = len(XSTAGE_ROWS)                    # 6


def _mk_nop(nc, engine):
    eng = nc.engines[engine]
    bi = eng.nop()
    inst = bi.ins
    bb = nc.cur_bb.bb
    lst = list(bb.instructions)
    assert lst and lst[-1].name == inst.name
    bb.instructions = lst[:-1]
    return inst


def fix_sync_overflow(nc, max_waits=1, max_updates=1):
    """This walrus build accepts one sync wait/update per instruction; split
    extras onto adjacent NOPs (same engine, program order preserves
    semantics)."""
    n_fix = 0
    for f in nc.m.functions:
        for bb in f.blocks:
            out = []
            changed = False
            for ins in bb.instructions:
                si = ins.sync_info
                if si is None:
                    out.append(ins)
                    continue
                waits = list(si.on_wait or [])
                updates = list(si.on_update or [])
                pre, post = [], []
                if len(waits) > max_waits:
                    extra, keep = waits[:-max_waits], waits[-max_waits:]
                    for w in extra:
                        nop = _mk_nop(nc, ins.engine)
                        nop.sync_info = mybir.SyncInfo(on_wait=[w], on_update=[])
                        pre.append(nop)
                    waits = keep
                    changed = True
                    n_fix += 1
                if len(updates) > max_updates:
                    keep, extra = updates[:max_updates], updates[max_updates:]
                    for u in extra:
                        nop = _mk_nop(nc, ins.engine)
                        nop.sync_info = mybir.SyncInfo(on_wait=[], on_update=[u])
                        post.append(nop)
                    updates = keep
                    changed = True
                    n_fix += 1
                if pre or post:
                    ins.sync_info = mybir.SyncInfo(on_wait=waits, on_update=updates)
                out.extend(pre)
                out.append(ins)
                out.extend(post)
            if changed:
                bb.instructions = out
    return n_fix


def _split_bf16(a, nterms):
    import ml_dtypes
    out = []
    r = np.asarray(a, np.float32)
    for _ in range(nterms):
        hi = r.astype(ml_dtypes.bfloat16)
        out.append(hi.astype(np.float32))
        r = (r - out[-1]).astype(np.float32)
    return out


def _geom_bias(b_out, b3, T):
    t = np.arange(1, T + 1, dtype=np.float64)
    if abs(1.0 - b3) < 1e-12:
        s = t.astype(np.float64)
    else:
        s = (1.0 - b3 ** t) / (1.0 - b3)
    return (b_out * s).astype(np.float32)


class Prep:
    """Host-side precomputation (shared by all cores)."""

    def __init__(self, W_in, b_in, beta_in, thr_in, W_h, b_h, beta_h, thr_h,
                 W_out, b_out, beta_out, T):
        f64 = np.float64
        self.T = T
        self.CB = min(128, T)              # layer-3 collection block
        self.nblk = (T + self.CB - 1) // self.CB
        k = K_BLK
        b1 = np.clip(beta_in.astype(f64), B_CLAMP, 1.0)
        b2 = np.clip(beta_h.astype(f64), B_CLAMP, 1.0)
        b3 = float(np.clip(beta_out.astype(f64), 0.0, 1.0)[0])
        thr1 = thr_in.astype(f64)
        thr2 = thr_h.astype(f64)
        win = W_in[:, 0].astype(f64)
        wout = W_out[0, :].astype(f64)

        s1 = np.stack([b1 ** (-j) for j in range(k)])      # [k,H]
        s2 = np.stack([b2 ** (-j) for j in range(k)])

        # cur1 combo lhsT_j [NXROW, H]
        self.cur1_lhsT = np.zeros((k, NXROW, H), np.float32)
        for j in range(k):
            A_t = _split_bf16((win * s1[j]).astype(np.float32), 3)
            for r, (a, xi) in enumerate(CUR1_PAIRS):
                self.cur1_lhsT[j, r] = A_t[a]

        # L1 spikes are +/-1 (ACT Sign): s1 = (g1+1)/2. W2 and reset1 operate
        # on g1 with halved coefficients; their constant halves are folded
        # into thresholds / rescale bias below.
        # W2_j: lhsT[k=h1, m=h2] = W_h[h2,h1]/2 * s2_j[h2]
        W2 = W_h.astype(f64).T[None, :, :] * s2[:, None, :] * 0.5
        self.W2_t = []
        for j in range(k):
            self.W2_t.append(_split_bf16(W2[j].astype(np.float32), NT_W2))
        # reset diags: L1 halved (g1 in +/-1), L2 plain (g2 in {0,1})
        self.d1_t, self.d2_t = [], []
        for j in range(k):
            self.d1_t.append([np.diag(v) for v in _split_bf16(
                (-(thr1 * s1[j]) * 0.5).astype(np.float32), NT_R1)])
            self.d2_t.append([np.diag(v) for v in _split_bf16(
                (-(thr2 * s2[j])).astype(np.float32), NT_R2)])

        # constant per-step inflows (folded, not matmul'd):
        beta1 = b_in.astype(f64) - 0.5 * thr1                 # [H]
        beta2 = b_h.astype(f64) + 0.5 * W_h.astype(f64).sum(axis=1)
        # D_j = sum_{i<=j} b^-i * beta  (missing accumulated bias at local j)
        D1 = np.cumsum(s1 * beta1[None, :], axis=0)           # [k,H]
        D2 = np.cumsum(s2 * beta2[None, :], axis=0)
        # effective thresholds c'_j = thr*b^-j - D_j
        c1p = thr1[None, :] * s1 - D1
        c2p = thr2[None, :] * s2 - D2
        self.c1n = (-c1p).astype(np.float32).T                # [H,k] Sign bias
        self.c2 = c2p.astype(np.float32).T                    # [H,k]
        self.r1 = (b1 ** k).astype(np.float32)[:, None]       # [H,1]
        self.r2 = (b2 ** k).astype(np.float32)[:, None]
        # rescale bias: restore the bias sum at block end
        self.rb1 = ((b1 ** k) * D1[k - 1]).astype(np.float32)[:, None]
        self.rb2 = ((b2 ** k) * D2[k - 1]).astype(np.float32)[:, None]

        # layer-3 Z buffers, even/odd parity so the sliding lhsT slice is
        # always 4-byte aligned in bf16.
        CB = self.CB
        wout_t = _split_bf16(wout.astype(np.float32), NT_ZW)
        self.Z_t, self.Zo_t = [], []
        for i in range(NT_ZW):
            Z = np.zeros((H, 2 * CB + 1), np.float32)
            Z[:, CB] = wout_t[i]
            self.Z_t.append(Z)                     # even w: slice CB-w (even)
            Zo = np.zeros((H, 2 * CB - 1), np.float32)
            Zo[:, CB - 1] = wout_t[i]
            self.Zo_t.append(Zo)                   # odd w: slice CB-1-w (even)

        # L-scan matrices (fp32) [CB, CB]
        idx = np.arange(CB)
        dt_ = idx[None, :] - idx[:, None]                     # t - tau
        with np.errstate(over='ignore', under='ignore'):
            LD = np.where(dt_ >= 0, b3 ** np.maximum(dt_, 0), 0.0)
        self.LD = LD.astype(np.float32)
        self.LF = []
        for d in range(1, self.nblk):
            with np.errstate(over='ignore', under='ignore'):
                M = b3 ** (dt_.astype(f64) + CB * d)
            M = np.where(np.isfinite(M), M, 0.0).astype(np.float32)
            self.LF.append(None if np.abs(M).max() < 1e-37 else M)
        self.l_bias = _geom_bias(float(np.asarray(b_out).ravel()[0]), b3,
                                 T).reshape(self.nblk, self.CB)

    def pack_params(self):
        import ml_dtypes
        k = K_BLK
        bf_cols, off_bf = [], {}

        def add_bf(name, arr2d):
            rows, C = arr2d.shape
            off_bf[name] = (sum(c.shape[1] for c in bf_cols), rows, C)
            a = np.zeros((128, C), np.float32)
            a[:rows] = arr2d
            bf_cols.append(a)

        for j in range(k):
            add_bf(f'cur1_{j}', self.cur1_lhsT[j])
        for i in range(NT_W2):
            for j in range(k):
                add_bf(f'w2_{i}_{j}', self.W2_t[j][i])
        for i in range(NT_R1):
            for j in range(k):
                add_bf(f'd1_{i}_{j}', self.d1_t[j][i])
        for i in range(NT_R2):
            for j in range(k):
                add_bf(f'd2_{i}_{j}', self.d2_t[j][i])
        for i in range(NT_ZW):
            add_bf(f'z_{i}', self.Z_t[i])
            add_bf(f'zo_{i}', self.Zo_t[i])
        bf16 = np.concatenate(bf_cols, axis=1).astype(ml_dtypes.bfloat16)

        f32_cols, off_f32 = [], {}

        def add_f32(name, arr2d):
            rows, C = arr2d.shape
            off_f32[name] = (sum(c.shape[1] for c in f32_cols), rows, C)
            a = np.zeros((128, C), np.float32)
            a[:rows] = arr2d
            f32_cols.append(a)

        add_f32('c1n', self.c1n)
        add_f32('c2', self.c2)
        add_f32('r1', self.r1)
        add_f32('r2', self.r2)
        add_f32('rb1', self.rb1)
        add_f32('rb2', self.rb2)
        add_f32('ld', self.LD)
        for d, M in enumerate(self.LF):
            if M is not None:
                add_f32(f'lf_{d + 1}', M)
        add_f32('lbias', self.l_bias.T.astype(np.float32))
        f32 = np.concatenate(f32_cols, axis=1).astype(np.float32)
        return bf16, f32, off_bf, off_f32


def stage_x(x_core):
    """x_core [T, B] f32 -> [NXROW, T*B] bf16 per XSTAGE_ROWS."""
    import ml_dtypes
    flat = x_core.reshape(-1).astype(np.float32)
    hi = flat.astype(ml_dtypes.bfloat16).astype(np.float32)
    r = flat - hi
    mid = r.astype(ml_dtypes.bfloat16).astype(np.float32)
    lo = (r - mid).astype(np.float32)
    rows = [hi, mid, lo]
    return np.stack([rows[i] for i in XSTAGE_ROWS]).astype(ml_dtypes.bfloat16)


def build_program(T, B_core, off_bf, off_f32, n_bf, n_f32, lf_present, CB, nblk):
    nc = bass.Bass(trn_type="TRN2", target_bir_lowering=False, debug=False,
                   num_devices=NCORES)
    k = K_BLK
    CHUNK = min(128, T)
    nchunk = T // CHUNK

    pbf_d = nc.dram_tensor("pbf", [128, n_bf], BF16, kind="ExternalInput").ap()
    pf_d = nc.dram_tensor("pf", [128, n_f32], F32, kind="ExternalInput").ap()
    xs_d = nc.dram_tensor("xs", [NXROW, T * B_core], BF16,
                          kind="ExternalInput").ap()
    y_d = nc.dram_tensor("y", [T, B_core], F32, kind="ExternalOutput").ap()

    with tile.TileContext(nc) as tc, ExitStack() as ctx:
        const = ctx.enter_context(tc.tile_pool(name="const", bufs=1))
        xpool = ctx.enter_context(tc.tile_pool(name="xpool", bufs=2))
        gpool = ctx.enter_context(tc.tile_pool(name="gpool", bufs=1))
        cpool = ctx.enter_context(tc.tile_pool(name="cpool", bufs=1))
        ypool = ctx.enter_context(tc.tile_pool(name="ypool", bufs=2))
        ps = ctx.enter_context(tc.tile_pool(name="ps", bufs=1, space="PSUM"))
        psL = ctx.enter_context(tc.tile_pool(name="psL", bufs=2, space="PSUM"))

        pbf = const.tile([128, n_bf], BF16)
        pf = const.tile([128, n_f32], F32)
        nc.sync.dma_start(pbf[:], pbf_d[:])
        nc.sync.dma_start(pf[:], pf_d[:])

        def bfp(name):
            o, rows, C = off_bf[name]
            return pbf[0:rows, o:o + C]

        def fpv(name, col, rows=128):
            o, _r, C = off_f32[name]
            return pf[0:rows, o + col:o + col + 1]

        def fpm(name):
            o, rows, C = off_f32[name]
            return pf[0:rows, o:o + C]

        P1 = ps.tile([128, B_core], F32, tag="P1")
        P2 = ps.tile([128, B_core], F32, tag="P2")
        Cb = ps.tile([128, B_core], F32, tag="Cb")

        g1 = [gpool.tile([128, B_core], BF16, tag=f"g1_{i}", name=f"g1_{i}")
              for i in range(2)]
        g2 = [gpool.tile([128, B_core], BF16, tag=f"g2_{i}", name=f"g2_{i}")
              for i in range(2)]
        nc.vector.memset(g1[1][:], -1.0)   # s1_prev=0 in +/-1 encoding
        nc.vector.memset(g2[1][:], 0.0)

        C_sb = [cpool.tile([128, B_core], F32, tag=f"csb_{i}", name=f"csb_{i}")
                for i in range(nblk)]

        xts = [xpool.tile([NXROW, CHUNK * B_core], BF16, tag="xt",
                          name=f"xt_{i}") for i in range(2)]
        nc.sync.dma_start(xts[0][:], xs_d[:, 0:CHUNK * B_core])
        if nchunk > 1:
            nc.sync.dma_start(xts[1][:],
                              xs_d[:, CHUNK * B_core:2 * CHUNK * B_core])

        def emit_zwin(tz):
            wz = tz % CB
            gz = g2[tz % 2]
            for i in range(NT_ZW):
                if wz % 2 == 0:
                    lhs = bfp(f'z_{i}')[:, CB - wz:2 * CB - wz]
                else:
                    lhs = bfp(f'zo_{i}')[:, CB - 1 - wz:2 * CB - 1 - wz]
                nc.tensor.matmul(Cb[0:CB, :], lhs, gz[:],
                                 start=(wz == 0 and i == 0), stop=False,
                                 skip_group_check=True)

        def emit_l2(t):
            # layer-2 of step t: reset (old g2) + W2 (g1_t) + spike2
            j = t % k
            cur = t % 2
            prv = 1 - cur
            for i in range(NT_R2):
                nc.tensor.matmul(P2[:], bfp(f'd2_{i}_{j}'), g2[prv][:],
                                 start=(t == 0 and i == 0), stop=False,
                                 skip_group_check=True)
            for i in range(NT_W2):
                nc.tensor.matmul(P2[:], bfp(f'w2_{i}_{j}'), g1[cur][:],
                                 start=False, stop=False,
                                 skip_group_check=True)
            nc.vector.tensor_scalar(g2[cur][:], P2[:], fpv('c2', j), None,
                                    op0=AOP.is_gt)
            if j == k - 1 and t != T - 1:
                nc.scalar.activation(P2[:], P2[:], AFT.Identity,
                                     bias=fpv('rb2', 0), scale=fpv('r2', 0))
            if t >= 1:
                emit_zwin(t - 1)
            if t % CB == CB - 1:
                emit_zwin(t)
                blk = t // CB
                nc.scalar.copy(C_sb[blk][:], Cb[:])
                nxt = t // CHUNK + 2
                if nxt < nchunk:
                    nc.sync.dma_start(
                        xts[nxt % 2][:],
                        xs_d[:, nxt * CHUNK * B_core:(nxt + 1) * CHUNK * B_core])

        for t in range(T):
            j = t % k
            cur = t % 2
            prv = 1 - cur
            xt = xts[(t // CHUNK) % 2]
            xo = (t % CHUNK) * B_core

            # ---- layer 1 of step t ----
            nc.tensor.matmul(P1[:], bfp(f'cur1_{j}'),
                             xt[0:NXROW, xo:xo + B_core],
                             start=(t == 0), stop=False, skip_group_check=True)
            for i in range(NT_R1):
                nc.tensor.matmul(P1[:], bfp(f'd1_{i}_{j}'), g1[prv][:],
                                 start=False, stop=False, skip_group_check=True)
            nc.scalar.activation(g1[cur][:], P1[:], AFT.Sign,
                                 bias=fpv('c1n', j), scale=1.0)
            if j == k - 1 and t != T - 1:
                nc.scalar.activation(P1[:], P1[:], AFT.Identity,
                                     bias=fpv('rb1', 0), scale=fpv('r1', 0))

            # ---- layer 2 of step t-1 (lagged emission: keeps the next P1
            #      group ahead of W2 in the PE stream) ----
            if t >= 1:
                emit_l2(t - 1)
        emit_l2(T - 1)

        # ---- L-scan phase ----
        for i in range(nblk):
            pl = psL.tile([128, B_core], F32, tag="pl", name=f"pl_{i}")
            nc.tensor.matmul(pl[0:CB, :], fpm('ld'), C_sb[i][0:CB, :],
                             start=True, stop=False, skip_group_check=True)
            for d in range(1, i + 1):
                if lf_present[d - 1]:
                    nc.tensor.matmul(pl[0:CB, :], fpm(f'lf_{d}'),
                                     C_sb[i - d][0:CB, :], start=False,
                                     stop=False, skip_group_check=True)
            ysb = ypool.tile([128, B_core], F32, tag="ysb", name=f"ysb_{i}")
            nc.scalar.activation(ysb[0:CB, :], pl[0:CB, :], AFT.Identity,
                                 bias=fpv('lbias', i, CB), scale=1.0)
            nc.sync.dma_start(y_d[CB * i:CB * (i + 1), :], ysb[0:CB, :])

    fix_sync_overflow(nc)
    return nc


_PROGRAM_CACHE = {}


def kernel(**inputs):
    x = np.asarray(inputs['x'], np.float32)
    T, B = x.shape[0], x.shape[1]
    B_core = B // NCORES
    x2 = x.reshape(T, B)

    prep = Prep(np.asarray(inputs['W_in'], np.float32),
                np.asarray(inputs['b_in'], np.float32),
                np.asarray(inputs['beta_in'], np.float32),
                np.asarray(inputs['thr_in'], np.float32),
                np.asarray(inputs['W_h'], np.float32),
                np.asarray(inputs['b_h'], np.float32),
                np.asarray(inputs['beta_h'], np.float32),
                np.asarray(inputs['thr_h'], np.float32),
                np.asarray(inputs['W_out'], np.float32),
                np.asarray(inputs['b_out'], np.float32),
                np.asarray(inputs['beta_out'], np.float32), T)
    pbf, pf, off_bf, off_f32 = prep.pack_params()
    lf_present = [M is not None for M in prep.LF]

    key = (T, B_core, pbf.shape[1], pf.shape[1], tuple(lf_present))
    if key not in _PROGRAM_CACHE:
        _PROGRAM_CACHE[key] = build_program(
            T, B_core, off_bf, off_f32, pbf.shape[1], pf.shape[1], lf_present,
            prep.CB, prep.nblk)
    nc = _PROGRAM_CACHE[key]

    in_maps = []
    for c in range(NCORES):
        xc = x2[:, c * B_core:(c + 1) * B_core]
        in_maps.append({'pbf': pbf, 'pf': pf, 'xs': stage_x(xc)})

    res = bass_utils.run_bass_kernel_spmd(nc, in_maps,
                                          core_ids=list(range(NCORES)))
    y = np.concatenate([res.results[c]['y'] for c in range(NCORES)], axis=1)
    return y.reshape(T, B, 1).astype(np.float32)
